# revision 30
# baseline (speedup 1.0000x reference)
"""TRN2 Bass kernel for nn_BasicAttn: per-batch attention
    P = keys[b] @ w            [Tk,K]@[K,V]   -> [Tk,V]
    L = P @ values[b].T        [Tk,V]@[V,Tv]  -> [Tk,Tv]
    A = softmax(L, axis=-1)
    O = A @ values[b]          [Tk,Tv]@[Tv,V] -> [Tk,V]
Returns (A, O) for all batches.

Sharding: data-parallel over batch B=16 across 8 cores (2 batches/core),
w replicated. Matmuls M1/M2 run in float32r (TF32-like, full PE rate),
M3 (A @ V) in bf16. Softmax in fp32.

Per-core structure (v2, software-pipelined):
  keys_nat [t,k] --PE transpose--> keysT [k,t] f32r     (M1 rhs)
  M1: P^T[v,t] = w[k,v]-weights.T @ keysT[k,t]          (accum k)
  valuesT [v,s] f32r via PE transpose; values_bf [s,d] bf16 cast
  M2: L[t,s] = P^T[v,t]-weights.T @ valuesT[v,s]        (accum v)
  softmax: DVE full-row max -> ACT exp(bias=-max, accum=sum)
  A^T via bf16 DMA xbar transpose (one 3D dma per t-tile)
  M3: O[t,d] = A^T[s,t]-weights.T @ values_bf[s,d]      (accum s)
  t-tiles pipelined one stage deep: M3/outputs of tile j-1 are
  emitted after M2 of tile j so PE hides the softmax latency.
"""

import os
import numpy as np

TV, B, V = 2048, 16, 1024
TK, K = 2048, 1024
NCORES = 8
BPC = B // NCORES  # batches per core

_CACHE = {}


def _build():
    if "nc" in _CACHE:
        return _CACHE

    import concourse.bacc as bacc
    import concourse.tile as tile
    from concourse import mybir
    from concourse.masks import make_identity

    f32 = mybir.dt.float32
    f32r = mybir.dt.float32r
    bf16 = mybir.dt.bfloat16
    AF = mybir.ActivationFunctionType
    AX = mybir.AxisListType
    OP = mybir.AluOpType

    nc = bacc.Bacc("TRN2", target_bir_lowering=False, dynamic_dma_scratch_size=2048)

    keys_d = nc.dram_tensor([BPC, TK, K], f32, kind="ExternalInput")
    values_d = nc.dram_tensor([TV, BPC, V], f32, kind="ExternalInput")
    w_d = nc.dram_tensor([K, V], f32, kind="ExternalInput")
    attn_d = nc.dram_tensor([BPC, TK, TV], f32, kind="ExternalOutput")
    out_d = nc.dram_tensor([BPC, TK, V], f32, kind="ExternalOutput")

    NS = TV // 128   # 16 s-tiles
    NV = V // 128    # 8 v-tiles
    NK = K // 128    # 8 k-tiles
    NTC = 4          # t-chunks of 512
    NSC = 4          # s-chunks of 512

    with tile.TileContext(nc) as tc:
        with (
            tc.tile_pool(name="const", bufs=1) as constp,
            tc.tile_pool(name="big", bufs=1) as big,
            tc.tile_pool(name="chunk", bufs=1) as chunkp,
            tc.tile_pool(name="nat", bufs=5) as natp,
            tc.tile_pool(name="work", bufs=2) as workp,
            tc.tile_pool(name="ebf", bufs=1) as ebfp,
            tc.tile_pool(name="atp", bufs=2) as atp,
            tc.tile_pool(name="osbp", bufs=1) as osbp,
            tc.tile_pool(name="wst", bufs=4) as wstp,
            tc.tile_pool(name="stat", bufs=4) as statp,
            tc.tile_pool(name="ps_a", bufs=2, space="PSUM") as ps_a,
            tc.tile_pool(name="ps_l", bufs=1, space="PSUM") as ps_l,
            tc.tile_pool(name="ps_o", bufs=1, space="PSUM") as ps_o,
        ):
            ident = constp.tile([128, 128], f32, tag="ident")
            make_identity(nc, ident[:])

            def evac(i, dst, src):
                # alternate PSUM evacuations between ACT and DVE
                if i % 2 == 0:
                    nc.scalar.copy(dst, src)
                else:
                    nc.vector.tensor_copy(dst, src)

            # ---- w: DMA straight into w_r (bitcast), round to f32r in place
            w_r = big.tile([128, NK, V], f32r, tag="w_r")

            def emit_w_load():
                # scalar-engine HWDGE queue: parallel to the SP load queue,
                # own small staging so the nat pool stays free for keys/values
                for kh in range(2 * NK):
                    k, h = kh // 2, kh % 2
                    wst = wstp.tile([128, 512], f32, tag="wst")
                    nc.scalar.dma_start(
                        wst[:],
                        w_d[k * 128:(k + 1) * 128, h * 512:(h + 1) * 512],
                    )
                    nc.scalar.copy(w_r[:, k, h * 512:(h + 1) * 512], wst[:])

            # one-stage software pipeline over t-tiles
            pending = []

            def emit_tail(ctx):
                b, t0, at, rd, expt = ctx
                Ops = ps_o.tile([128, V], f32, tag="ps_o")
                for st in range(NS):
                    for dc in range(2):
                        nc.tensor.matmul(
                            Ops[:, dc * 512:(dc + 1) * 512],
                            at[:, st, :],
                            values_bf[:, st, dc * 512:(dc + 1) * 512],
                            start=(st == 0),
                            stop=(st == NS - 1),
                        )
                osb = osbp.tile([128, V], f32, tag="osb")
                nc.vector.tensor_scalar_mul(osb[:], Ops[:], rd[:])
                nc.sync.dma_start(out_d[b, t0:t0 + 128, :], osb[:])
                # normalize attn in place (cast to bf16 already done)
                nc.vector.tensor_scalar_mul(expt[:], expt[:], rd[:])
                nc.sync.dma_start(attn_d[b, t0:t0 + 128, :], expt[:])

            def flush():
                while pending:
                    emit_tail(pending.pop(0))

            def emit_T_tile(nat, dst, col0, ei):
                # transpose one [128,1024] natural tile into 8 [128,128]
                # blocks of `dst` ([128, 8, *]) at column offset col0.
                # Two PSUM banks, each evacuated with one 3D strided copy.
                for h in range(2):
                    pst = ps_a.tile([128, 512], f32, tag="ps_a")
                    for u in range(4):
                        sl = 4 * h + u
                        nc.tensor.transpose(
                            pst[:, u * 128:(u + 1) * 128],
                            nat[:, sl * 128:(sl + 1) * 128],
                            ident[:],
                        )
                    evac(
                        ei + h,
                        dst[:, 4 * h:4 * h + 4, col0:col0 + 128],
                        pst[:].rearrange("p (a x) -> p a x", a=4),
                    )

            def emit_k_loads(b, tch):
                knats = []
                for j in range(4):
                    tt = tch * 4 + j
                    knat = natp.tile([128, 1024], f32, tag="nat")
                    nc.sync.dma_start(
                        knat[:], keys_d[b, tt * 128:(tt + 1) * 128, :]
                    )
                    knats.append(knat)
                return knats

            def emit_kT(knats, keysT):
                for j, knat in enumerate(knats):
                    emit_T_tile(knat, keysT, j * 128, j)

            def emit_m1(keysT, PT, v):
                psm = ps_a.tile([128, 512], f32, tag="ps_a")
                for k in range(NK):
                    nc.tensor.matmul(
                        psm[:],
                        w_r[:, k, v * 128:(v + 1) * 128],
                        keysT[:, k, :],
                        start=(k == 0),
                        stop=(k == NK - 1),
                    )
                evac(v, PT[:, v, :], psm[:])

            emit_w_load()
            prefetched = None
            for b in range(BPC):
                flush()
                valuesT = big.tile([128, NV, TV], f32r, tag="valuesT")
                values_bf = big.tile([128, NS, V], bf16, tag="values_bf")
                # keys chunk 0 first: gives PE dense work while values stream
                keysT = chunkp.tile([128, NK, 512], f32r, tag="keysT")
                PT = chunkp.tile([128, NV, 512], f32r, tag="PT")
                knats = prefetched if prefetched is not None else emit_k_loads(b, 0)
                prefetched = None
                emit_kT(knats, keysT)
                # values phase; M1 after sc0 (w still streaming in), t-tile 0's
                # M2 chunks interleave to keep the PE on real matmul work
                def emit_m2_chunk(Lc, jloc, c):
                    for v in range(NV):
                        nc.tensor.matmul(
                            Lc,
                            PT[:, v, jloc * 128:(jloc + 1) * 128],
                            valuesT[:, v, c * 512:(c + 1) * 512],
                            start=(v == 0),
                            stop=(v == NV - 1),
                        )

                Lps0 = ps_l.tile([128, TV], f32, tag="ps_l")
                for sc in range(NSC):
                    for j in range(4):
                        st = sc * 4 + j
                        vnat = natp.tile([128, 1024], f32, tag="nat")
                        nc.sync.dma_start(
                            vnat[:], values_d[st * 128:(st + 1) * 128, b, :]
                        )
                        nc.vector.tensor_copy(values_bf[:, st, :], vnat[:])
                        emit_T_tile(vnat, valuesT, st * 128, st)
                    if b == 0 and sc == NSC - 1:
                        # batch 0: w arrives late; M1 after the values phase
                        for v in range(NV):
                            emit_m1(keysT, PT, v)
                    if b > 0 and sc == 0:
                        # later batches: w resident; M1 early, M2 interleaves
                        for v in range(NV):
                            emit_m1(keysT, PT, v)
                    if b > 0 and sc >= 1:
                        c = sc - 1
                        emit_m2_chunk(Lps0[:, c * 512:(c + 1) * 512], 0, c)
                if b > 0:
                    emit_m2_chunk(Lps0[:, 3 * 512:4 * 512], 0, 3)
                else:
                    for c in range(NSC):
                        emit_m2_chunk(Lps0[:, c * 512:(c + 1) * 512], 0, c)

                for tch in range(NTC):
                    if tch > 0:
                        # ---- keys chunk: transpose prefetched tiles, M1
                        keysT = chunkp.tile([128, NK, 512], f32r, tag="keysT")
                        emit_kT(prefetched, keysT)
                        prefetched = None
                        PT = chunkp.tile([128, NV, 512], f32r, tag="PT")
                        for v in range(NV):
                            emit_m1(keysT, PT, v)

                    # prefetch next chunk's (or next batch's) keys
                    if tch < NTC - 1:
                        prefetched = emit_k_loads(b, tch + 1)
                    elif b + 1 < BPC:
                        prefetched = emit_k_loads(b + 1, 0)

                    # ---- per t-tile: M2 logits -> softmax -> A^T (dma), then
                    # tail (M3 + outputs) of the previous t-tile
                    for j in range(4):
                        tt = tch * 4 + j
                        t0 = tt * 128
                        if tch == 0 and j == 0:
                            Lps = Lps0
                        else:
                            Lps = ps_l.tile([128, TV], f32, tag="ps_l")
                            for c in range(NSC):
                                emit_m2_chunk(
                                    Lps[:, c * 512:(c + 1) * 512], j, c
                                )
                        negm = statp.tile([128, 1], f32, tag="negm")
                        nc.vector.tensor_reduce(
                            negm[:], Lps[:], axis=AX.X, op=OP.max, negate=True
                        )
                        expt = workp.tile([128, TV], f32, tag="expt")
                        dsum = statp.tile([128, 1], f32, tag="dsum")
                        nc.scalar.activation(
                            expt[:], Lps[:], AF.Exp,
                            bias=negm[:], scale=1.0, accum_out=dsum[:],
                        )
                        rd = statp.tile([128, 1], f32, tag="rd")
                        nc.vector.reciprocal(rd[:], dsum[:])
                        ebf = ebfp.tile([128, TV], bf16, tag="ebf")
                        nc.vector.tensor_copy(ebf[:], expt[:])
                        at = atp.tile([128, NS, 128], bf16, tag="at")
                        nc.scalar.dma_start_transpose(at[:], ebf[:])

                        emit_tail_ctx = (b, t0, at, rd, expt)
                        pending.append(emit_tail_ctx)
                        if len(pending) > 1:
                            emit_tail(pending.pop(0))

            flush()

    nc.finalize()
    _CACHE.update(
        nc=nc,
        names=dict(
            keys=keys_d.name, values=values_d.name, w=w_d.name,
            attn=attn_d.name, out=out_d.name,
        ),
    )
    return _CACHE


def kernel(values, values_mask, keys, w):
    from concourse.bass_utils import run_bass_kernel_spmd

    c = _build()
    nc, names = c["nc"], c["names"]

    values = np.asarray(values, dtype=np.float32)
    keys = np.asarray(keys, dtype=np.float32)
    w = np.ascontiguousarray(np.asarray(w, dtype=np.float32))

    in_maps = []
    for core in range(NCORES):
        b0 = core * BPC
        in_maps.append({
            names["keys"]: np.ascontiguousarray(keys[b0:b0 + BPC]),
            names["values"]: np.ascontiguousarray(values[:, b0:b0 + BPC, :]),
            names["w"]: w,
        })

    trace = bool(int(os.environ.get("BASS_KERNEL_TRACE", "0")))
    res = run_bass_kernel_spmd(
        nc, in_maps, core_ids=list(range(NCORES)), trace=trace,
    )
    _CACHE["last_results"] = res

    attn = np.concatenate([r[names["attn"]] for r in res.results], axis=0)
    outp = np.concatenate([r[names["out"]] for r in res.results], axis=0)
    return attn, outp
